# revision 13
# baseline (speedup 1.0000x reference)
"""AttentionNetPooling on 8 Trainium2 NeuronCores.

Math (see reference): scores = MLP(z); weights = softmax(scores) over ALL
nodes; out[g] = sum_{i in g} weights[i] * z[i, :256] / count[g].

Design (evolved from trace analysis):
 - PE HAM warm-up: ~5us of dummy accumulating matmuls emitted before any
   real work so the PE clock-gate opens (K=8/8, 2.4GHz) before the real
   stream starts; the fused loop then has no >=3.4us PE gap.
 - One fused streaming loop over 2048-node quads (software pipeline:
   scores(k-1) / MLP(k) / segment-matmuls(k-1)), so the PE, ACT, DVE and
   DMA all stay busy together; there is no separate phase 2.
 - Chunk-major DRAM layouts: each quad's data is one contiguous DRAM
   block (the DMA engines sweep it together at ~25B/ns/engine; a
   partition-major layout measured only ~15.8B/ns due to HBM row
   thrash).
 - Softmax denominator finished on host: the unmasked w_all matrix is
   DMAed out; the host zeroes the pad entries (it knows the plan),
   computes S = sum over cores, and divides by (S * count[g]) during the
   gather.  No partition_all_reduce, no cross-core AllReduce, no mask
   multiply on device (pads never reach the one-hot: colidx=-1 kills
   them there).
 - Window-0 outputs (per-graph sums and w_all columns) are DMAed out
   mid-loop so only window-1's small DMAs trail the last matmul.

Host: partition graphs into 8 contiguous ranges balanced by node count;
each core's range splits into 2 windows of <=128 graphs; each window's
node span is zero-padded to a common tile count (SPMD-identical
program).  Host pre-swizzles z into feature-major bf16 (MLP contraction)
and node-major bf16 of the pooled 256 columns (segment matmul).
"""
import numpy as np
import ml_dtypes

import concourse.bass as bass
import concourse.bacc as bacc
import concourse.tile as tile
import concourse.mybir as mybir
from concourse.bass_utils import run_bass_kernel_spmd

F32 = mybir.dt.float32
BF16 = mybir.dt.bfloat16
AF = mybir.ActivationFunctionType
ALU = mybir.AluOpType

NCORES = 8
P = 128           # partitions / nodes per tile
IN_DIM = 320
POOL = 256
HID = 128
SCG = 4           # superchunks (512 nodes) per pipeline group
NWARM = 13        # dummy 256-col matmuls; ~5us cold, enough for the HAM

# test.py hooks: set trace=True to NTFF-profile; LAST_RESULT holds the
# BassKernelResults of the most recent kernel() call.
PROFILE = {"trace": False, "tmpdir": None}
LAST_RESULT = None

_BUILD_CACHE = {}


def _plan(batch_index, num_graphs):
    """Partition graphs into 8 node-balanced contiguous ranges, split each
    into 2 windows of <=128 graphs, pad window node spans to shared tile
    counts T0/T1 (each a multiple of 4 tiles = 512-node superchunks)."""
    G = int(num_graphs)
    N = batch_index.shape[0]
    counts = np.bincount(batch_index, minlength=G).astype(np.int64)
    cum = np.concatenate([[0], np.cumsum(counts)])  # cum[g] = first node of g

    bounds = [0]
    for c in range(1, NCORES):
        g = int(np.searchsorted(cum, c * N / NCORES))
        g = max(bounds[-1], min(g, G))
        g = max(g, G - 256 * (NCORES - c))   # leave <=256 per remaining core
        g = min(g, bounds[-1] + 256)
        bounds.append(g)
    bounds.append(G)

    cores = []
    for c in range(NCORES):
        g_lo, g_hi = bounds[c], bounds[c + 1]
        assert g_hi - g_lo <= 256
        half = (cum[g_lo] + cum[g_hi]) / 2
        m = int(np.searchsorted(cum, half))
        m = max(g_lo, min(m, g_lo + 128))
        m = max(m, g_hi - 128)
        m = min(m, g_hi)
        wins = []
        for a, b in ((g_lo, m), (m, g_hi)):
            wins.append({"g_lo": a, "g_hi": b,
                         "n_lo": int(cum[a]), "n_hi": int(cum[b])})
        cores.append(wins)

    T = [4 * max(1, -(-max(cores[c][w]["n_hi"] - cores[c][w]["n_lo"]
                        for c in range(NCORES)) // 512)) for w in range(2)]
    return counts, cores, T


def _build_inputs(z, batch_index, W1, b1, W2, b2, counts, cores, T):
    nT = T[0] + T[1]
    nCh = nT // 2
    nSc = nT // 4
    Npad = nT * P
    b2s = float(np.asarray(b2).reshape(-1)[0])

    # shared constants
    W1T = np.zeros((P, 384), dtype=ml_dtypes.bfloat16)  # [k-in-chunk, 128c + h]
    w1t = np.ascontiguousarray(W1.T)            # [320, 128]
    for ch in range(3):
        k0, k1 = 128 * ch, min(128 * (ch + 1), IN_DIM)
        W1T[: k1 - k0, 128 * ch: 128 * ch + HID] = w1t[k0:k1]
    W2T = np.ascontiguousarray(
        W2.reshape(1, HID).T).astype(ml_dtypes.bfloat16)   # [128, 1]
    b1c = np.asarray(b1, dtype=np.float32).reshape(HID, 1)
    iota = np.tile(np.arange(P, dtype=ml_dtypes.bfloat16), (P, 1))

    in_maps = []
    host_masks = []
    for c in range(NCORES):
        zp = np.zeros((Npad, IN_DIM), dtype=np.float32)
        colidx = np.full(Npad, -1.0, dtype=np.float32)
        mask = np.zeros(Npad, dtype=np.float32)
        for w in range(2):
            win = cores[c][w]
            base = T[0] * P if w else 0
            n = win["n_hi"] - win["n_lo"]
            zp[base: base + n] = z[win["n_lo"]: win["n_hi"]]
            colidx[base: base + n] = (
                batch_index[win["n_lo"]: win["n_hi"]] - win["g_lo"]
            ).astype(np.float32)
            mask[base: base + n] = 1.0

        # feature-major swizzle for the MLP (512-node superchunks):
        # zth01[s, p, 512c+j] = zp[512s+j, 128c+p]; zth2 = features 256:320
        zq = zp.reshape(nSc, 512, IN_DIM).transpose(0, 2, 1)  # s, f, j
        zth01 = np.ascontiguousarray(
            zq[:, 0:256].reshape(nSc, 2, P, 512).transpose(0, 2, 1, 3)
            .reshape(nSc, P, 1024)).astype(ml_dtypes.bfloat16)
        zth2 = np.ascontiguousarray(
            zq[:, 256:320]).astype(ml_dtypes.bfloat16)        # [nSc, 64, 512]

        # node-major bf16 of pooled columns for the segment matmul,
        # superchunk-major so the quad DMA reads 2KB runs per partition:
        # znm[s, p, 512c+256j+d] = zp[512s+256c+128j+p, d]
        znm = zp[:, :POOL].reshape(nSc, 2, 2, P, POOL).transpose(
            0, 3, 1, 2, 4).reshape(nSc, P, 1024).astype(ml_dtypes.bfloat16)

        in_maps.append({
            "zth01": zth01, "zth2": zth2, "znm": np.ascontiguousarray(znm),
            "colidx": np.ascontiguousarray(colidx.reshape(nT, P).T),
            "w1t": W1T, "w2t": W2T, "b1": b1c, "iota": iota,
            "b2s": np.full((P, 1), b2s, dtype=np.float32),
        })
        host_masks.append(np.ascontiguousarray(mask.reshape(nT, P).T))
    return in_maps, host_masks


def _build_program(T):
    key = tuple(T)
    if key in _BUILD_CACHE:
        return _BUILD_CACHE[key]
    nT = T[0] + T[1]
    nCh = nT // 2
    nSc = nT // 4
    nGrp = -(-nSc // SCG)

    nc = bacc.Bacc("TRN2", target_bir_lowering=False, debug=False,
                   num_devices=NCORES)
    zth01_d = nc.dram_tensor("zth01", [nSc, P, 1024], BF16,
                             kind="ExternalInput").ap()
    zth2_d = nc.dram_tensor("zth2", [nSc, 64, 512], BF16,
                            kind="ExternalInput").ap()
    znm_d = nc.dram_tensor("znm", [nSc, P, 1024], BF16,
                           kind="ExternalInput").ap()
    colidx_d = nc.dram_tensor("colidx", [P, nT], F32, kind="ExternalInput").ap()
    b2s_d = nc.dram_tensor("b2s", [P, 1], F32, kind="ExternalInput").ap()
    w1t_d = nc.dram_tensor("w1t", [P, 384], BF16, kind="ExternalInput").ap()
    w2t_d = nc.dram_tensor("w2t", [HID, 1], BF16, kind="ExternalInput").ap()
    b1_d = nc.dram_tensor("b1", [HID, 1], F32, kind="ExternalInput").ap()
    iota_d = nc.dram_tensor("iota", [P, P], BF16, kind="ExternalInput").ap()
    out_d = nc.dram_tensor("out", [2 * P, POOL], F32, kind="ExternalOutput").ap()
    wall_d = nc.dram_tensor("wall", [P, nT], F32, kind="ExternalOutput").ap()

    with tile.TileContext(nc) as tc:
        with tc.tile_pool(name="const", bufs=1) as cpool, \
             tc.tile_pool(name="zth", bufs=6) as zthpool, \
             tc.tile_pool(name="znmp", bufs=6) as znmpool, \
             tc.tile_pool(name="hs", bufs=8) as hspool, \
             tc.tile_pool(name="oh", bufs=6) as ohpool, \
             tc.tile_pool(name="fin", bufs=1) as fpool, \
             tc.tile_pool(name="ps_h", bufs=SCG, space="PSUM") as psh, \
             tc.tile_pool(name="ps_s", bufs=2, space="PSUM") as pss, \
             tc.tile_pool(name="ps_B", bufs=1, space="PSUM") as psB:

            nch = [T[0] // 2, T[1] // 2]
            Bacc = [[psB.tile([P, POOL], F32, tag=f"B{w}a{a}",
                              name=f"B{w}a{a}")
                     for a in range(1)] for w in range(2)]

            # ---- HAM warm-up: ~5us of dummy accumulating matmuls on a
            # zeroed SBUF tile so the PE clock-gate opens before the real
            # stream; the garbage lands in Bacc[0][0], whose first real
            # matmul (start=True) resets it. ----
            warm_sb = cpool.tile([P, 256], BF16)
            nc.gpsimd.memset(warm_sb[:], 0.0)
            for i in range(NWARM):
                nc.tensor.matmul(Bacc[0][0][:], warm_sb[:, 0:128], warm_sb[:],
                                 start=(i == 0), stop=(i == NWARM - 1))

            # ---- constants ----
            w1t_sb = cpool.tile([P, 384], BF16)
            nc.gpsimd.dma_start(w1t_sb[:], w1t_d[:])
            w2t_sb = cpool.tile([HID, 1], BF16)
            nc.gpsimd.dma_start(w2t_sb[:], w2t_d[:])
            b1_sb = cpool.tile([HID, 1], F32)
            nc.gpsimd.dma_start(b1_sb[:], b1_d[:])
            iota_sb = cpool.tile([P, P], BF16)
            nc.gpsimd.dma_start(iota_sb[:], iota_d[:])
            colidx_sb = cpool.tile([P, nT], F32)
            nc.gpsimd.dma_start(colidx_sb[:], colidx_d[:])
            b2s_sb = cpool.tile([P, 1], F32)
            nc.gpsimd.dma_start(b2s_sb[:], b2s_d[:])
            w_all = cpool.tile([P, nT], F32)

            def dma_grp(k):
                sq0 = SCG * k
                np_ = min(SCG, nSc - sq0)
                zthp = zthpool.tile([P, np_ * 1024], BF16, tag="zthp",
                                    name="zthp")
                nc.sync.dma_start(
                    zthp[:].rearrange("p (a b) -> p a b", a=np_),
                    zth01_d[sq0: sq0 + np_].rearrange("a p b -> p a b"))
                zth2p = zthpool.tile([64, np_ * 512], BF16, tag="zth2p",
                                     name="zth2p")
                nc.sync.dma_start(
                    zth2p[:].rearrange("p (a b) -> p a b", a=np_),
                    zth2_d[sq0: sq0 + np_].rearrange("a p b -> p a b"))
                znmp = znmpool.tile([P, np_ * 1024], BF16, tag="znmp",
                                    name="znmp")
                nc.gpsimd.dma_start(
                    znmp[:].rearrange("p (a b) -> p a b", a=np_),
                    znm_d[sq0: sq0 + np_].rearrange("a p b -> p a b"))
                return zthp, zth2p, znmp

            def drain_window(w):
                # copy the window's PSUM accumulators to SBUF and DMA out
                accs = Bacc[w]
                tot = fpool.tile([P, POOL], F32, tag=f"comb{w}",
                                 name=f"comb{w}")
                nc.vector.tensor_copy(tot[:], accs[0][:])
                for a in range(1, len(accs)):
                    nc.vector.tensor_tensor(tot[:], tot[:], accs[a][:],
                                            ALU.add)
                nc.sync.dma_start(out_d[P * w: P * (w + 1), :], tot[:])

            bufs = {0: dma_grp(0)}
            if nGrp > 1:
                bufs[1] = dma_grp(1)

            drained = [False, False]
            wall_sent = [False, False]
            prev = None  # (superchunk ids, hs tiles, znmp tile, group idx)
            for k in range(nGrp + 1):
                if k < nGrp:
                    if k + 2 < nGrp:
                        bufs[k + 2] = dma_grp(k + 2)
                    zthp, zth2p, znmp = bufs.pop(k)
                    sq0 = SCG * k
                    np_ = min(SCG, nSc - sq0)

                # ---- scores + w = exp(s+b2) for the previous group ----
                if prev is not None:
                    psqs, phs, _, _ = prev
                    for idx, sq in enumerate(psqs):
                        s_ps = pss.tile([P, 4], F32, tag="s",
                                        name=f"sps{sq % 2}")
                        for j in range(4):
                            nc.tensor.matmul(
                                s_ps[:, j: j + 1],
                                phs[idx][:, 128 * j: 128 * (j + 1)],
                                w2t_sb[:], start=True, stop=True)
                        with tc.high_priority(offset=64):
                            nc.scalar.activation(w_all[:, 4 * sq: 4 * sq + 4],
                                                 s_ps[:], AF.Exp,
                                                 bias=b2s_sb[:])
                    # window-0 w columns all written -> ship them mid-loop
                    if psqs[-1] >= T[0] // 4 - 1 and not wall_sent[0]:
                        nc.sync.dma_start(wall_d[:, 0: T[0]],
                                          w_all[:, 0: T[0]])
                        wall_sent[0] = True

                # ---- MLP for group k ----
                if k < nGrp:
                    hps = [psh.tile([P, 512], F32, tag="h",
                                    name=f"hps{(sq0 + i) % SCG}")
                           for i in range(np_)]
                    for ch, a, b in ((0, 0, 512), (1, 512, 1024)):
                        for i in range(np_):
                            nc.tensor.matmul(
                                hps[i][:], w1t_sb[:, 128 * ch: 128 * (ch + 1)],
                                zthp[:, 1024 * i + a: 1024 * i + b],
                                start=(ch == 0), stop=False)
                    for i in range(np_):
                        nc.tensor.matmul(hps[i][:], w1t_sb[0:64, 256:384],
                                         zth2p[:, 512 * i: 512 * (i + 1)],
                                         start=False, stop=True)
                    hs = []
                    for i in range(np_):
                        h_sb = hspool.tile([P, 512], BF16, tag="hs",
                                           name=f"hs{(sq0 + i) % SCG}")
                        with tc.high_priority(offset=64):
                            nc.scalar.activation(h_sb[:], hps[i][:], AF.Relu,
                                                 bias=b1_sb[:])
                        hs.append(h_sb)

                # ---- one-hot + segment matmuls for the previous group ----
                if prev is not None:
                    psqs, phs, pznmp, pk = prev
                    for lq in range(2 * len(psqs)):
                        q = 2 * SCG * pk + lq
                        w = 0 if q < T[0] // 2 else 1
                        qw = q - (T[0] // 2 if w else 0)
                        oh = ohpool.tile([P, 256], BF16, tag="oh", name="oh")
                        for j in (0, 1):
                            t = 2 * q + j
                            with tc.high_priority(offset=64):
                                nc.vector.tensor_scalar(
                                    oh[:, 128 * j: 128 * (j + 1)], iota_sb[:],
                                    colidx_sb[:, t: t + 1], w_all[:, t: t + 1],
                                    ALU.is_equal, ALU.mult)
                        na = len(Bacc[w])
                        acc = Bacc[w][qw % na]
                        sl, c2 = lq // 2, lq % 2
                        zo = 1024 * sl + 512 * c2
                        for j in (0, 1):
                            nc.tensor.matmul(
                                acc[:], oh[:, 128 * j: 128 * (j + 1)],
                                pznmp[:, zo + 256 * j: zo + 256 * (j + 1)],
                                start=(qw < na and j == 0),
                                stop=(qw + na >= nch[w] and j == 1))
                        # window finished -> ship its sums mid-loop
                        if qw == nch[w] - 1 and not drained[w]:
                            drain_window(w)
                            drained[w] = True

                # keep-warm fillers: the loop is DMA-bound (~8us/group)
                # while the warm PE only needs ~5.5us, so without filler
                # the PE idles >3.4us and the HAM re-throttles it to
                # 1.2GHz (which then makes the PE the bottleneck).  Dummy
                # matmuls plug the idle below the HAM window.  They land
                # in whichever window accumulator is not live: window 1's
                # before its first real chunk, window 0's after its drain.
                g_w1_first = (T[0] // 2) // (2 * SCG)      # group of w1's 1st chunk
                g_w0_last = (T[0] // 2 - 1) // (2 * SCG)   # group of w0's last chunk
                nfill = 24
                fb = None
                if 1 <= k <= g_w1_first:       # seg processed here is group k-1
                    fb = Bacc[1][0]
                elif g_w0_last + 2 <= k < nGrp:  # w0 drained in iter g_w0_last+1
                    fb = Bacc[0][0]
                if fb is not None and T[0] // 2 > 2 * SCG * 3:
                    for i in range(nfill):
                        nc.tensor.matmul(fb[:], warm_sb[:, 0:128],
                                         warm_sb[:], start=(i == 0),
                                         stop=(i == nfill - 1))

                if k < nGrp:
                    prev = ([sq0 + i for i in range(np_)], hs, znmp, k)
                else:
                    prev = None

            # ---- remaining w columns (window 1) ----
            nc.sync.dma_start(wall_d[:, T[0]: nT], w_all[:, T[0]: nT])
            for w in range(2):
                if not drained[w]:
                    drain_window(w)

    nc.compile()
    _BUILD_CACHE[key] = nc
    return nc


def kernel(z, batch_index, W1, b1, W2, b2, num_graphs):
    global LAST_RESULT
    z = np.asarray(z, dtype=np.float32)
    batch_index = np.asarray(batch_index)
    G = int(num_graphs)

    counts, cores, T = _plan(batch_index, G)
    in_maps, host_masks = _build_inputs(
        z, batch_index, np.asarray(W1), np.asarray(b1),
        np.asarray(W2), np.asarray(b2), counts, cores, T)
    nc = _build_program(T)

    res = run_bass_kernel_spmd(
        nc, in_maps, list(range(NCORES)),
        trace=PROFILE["trace"],
        **({"tmpdir": PROFILE["tmpdir"]} if PROFILE["tmpdir"] else {}))
    LAST_RESULT = res

    # host-side finish: global softmax denominator (pads masked here) and
    # the per-graph scaling by 1/(S * count)
    S = 0.0
    for c in range(NCORES):
        wall = np.asarray(res.results[c]["wall"], np.float64)
        S += float((wall * host_masks[c]).sum())
    out = np.zeros((G, POOL), dtype=np.float32)
    for c in range(NCORES):
        for w in range(2):
            win = cores[c][w]
            ng = win["g_hi"] - win["g_lo"]
            if ng:
                B = np.asarray(res.results[c]["out"][P * w: P * w + ng],
                               np.float64)
                cnt = np.maximum(counts[win["g_lo"]: win["g_hi"]],
                                 1).astype(np.float64)
                out[win["g_lo"]: win["g_hi"]] = \
                    (B / (S * cnt[:, None])).astype(np.float32)
    return out


# revision 14
# speedup vs baseline: 1.0516x; 1.0516x over previous
"""AttentionNetPooling on 8 Trainium2 NeuronCores.

Math (see reference): scores = MLP(z); weights = softmax(scores) over ALL
nodes; out[g] = sum_{i in g} weights[i] * z[i, :256] / count[g].

Design (evolved from trace analysis):
 - PE HAM warm-up: ~5us of dummy accumulating matmuls emitted before any
   real work so the PE clock-gate opens (K=8/8, 2.4GHz) before the real
   stream starts; the fused loop then has no >=3.4us PE gap.
 - One fused streaming loop over 2048-node quads (software pipeline:
   scores(k-1) / MLP(k) / segment-matmuls(k-1)), so the PE, ACT, DVE and
   DMA all stay busy together; there is no separate phase 2.
 - Chunk-major DRAM layouts: each quad's data is one contiguous DRAM
   block (the DMA engines sweep it together at ~25B/ns/engine; a
   partition-major layout measured only ~15.8B/ns due to HBM row
   thrash).
 - Softmax denominator finished on host: the unmasked w_all matrix is
   DMAed out; the host zeroes the pad entries (it knows the plan),
   computes S = sum over cores, and divides by (S * count[g]) during the
   gather.  No partition_all_reduce, no cross-core AllReduce, no mask
   multiply on device (pads never reach the one-hot: colidx=-1 kills
   them there).
 - Window-0 outputs (per-graph sums and w_all columns) are DMAed out
   mid-loop so only window-1's small DMAs trail the last matmul.

Host: partition graphs into 8 contiguous ranges balanced by node count;
each core's range splits into 2 windows of <=128 graphs; each window's
node span is zero-padded to a common tile count (SPMD-identical
program).  Host pre-swizzles z into feature-major bf16 (MLP contraction)
and node-major bf16 of the pooled 256 columns (segment matmul).
"""
import numpy as np
import ml_dtypes

import concourse.bass as bass
import concourse.bacc as bacc
import concourse.tile as tile
import concourse.mybir as mybir
from concourse.bass_utils import run_bass_kernel_spmd

F32 = mybir.dt.float32
BF16 = mybir.dt.bfloat16
AF = mybir.ActivationFunctionType
ALU = mybir.AluOpType

NCORES = 8
P = 128           # partitions / nodes per tile
IN_DIM = 320
POOL = 256
HID = 128
SCG = 4           # superchunks (512 nodes) per pipeline group
NWARM = 13        # dummy 256-col matmuls; ~5us cold, enough for the HAM

# test.py hooks: set trace=True to NTFF-profile; LAST_RESULT holds the
# BassKernelResults of the most recent kernel() call.
PROFILE = {"trace": False, "tmpdir": None}
LAST_RESULT = None

_BUILD_CACHE = {}


def _plan(batch_index, num_graphs):
    """Partition graphs into 8 node-balanced contiguous ranges, split each
    into 2 windows of <=128 graphs, pad window node spans to shared tile
    counts T0/T1 (each a multiple of 4 tiles = 512-node superchunks)."""
    G = int(num_graphs)
    N = batch_index.shape[0]
    counts = np.bincount(batch_index, minlength=G).astype(np.int64)
    cum = np.concatenate([[0], np.cumsum(counts)])  # cum[g] = first node of g

    bounds = [0]
    for c in range(1, NCORES):
        g = int(np.searchsorted(cum, c * N / NCORES))
        g = max(bounds[-1], min(g, G))
        g = max(g, G - 256 * (NCORES - c))   # leave <=256 per remaining core
        g = min(g, bounds[-1] + 256)
        bounds.append(g)
    bounds.append(G)

    cores = []
    for c in range(NCORES):
        g_lo, g_hi = bounds[c], bounds[c + 1]
        assert g_hi - g_lo <= 256
        half = (cum[g_lo] + cum[g_hi]) / 2
        m = int(np.searchsorted(cum, half))
        m = max(g_lo, min(m, g_lo + 128))
        m = max(m, g_hi - 128)
        m = min(m, g_hi)
        wins = []
        for a, b in ((g_lo, m), (m, g_hi)):
            wins.append({"g_lo": a, "g_hi": b,
                         "n_lo": int(cum[a]), "n_hi": int(cum[b])})
        cores.append(wins)

    T = [4 * max(1, -(-max(cores[c][w]["n_hi"] - cores[c][w]["n_lo"]
                        for c in range(NCORES)) // 512)) for w in range(2)]
    return counts, cores, T


def _build_inputs(z, batch_index, W1, b1, W2, b2, counts, cores, T):
    nT = T[0] + T[1]
    nCh = nT // 2
    nSc = nT // 4
    Npad = nT * P
    b2s = float(np.asarray(b2).reshape(-1)[0])

    # shared constants
    W1T = np.zeros((P, 384), dtype=ml_dtypes.bfloat16)  # [k-in-chunk, 128c + h]
    w1t = np.ascontiguousarray(W1.T)            # [320, 128]
    for ch in range(3):
        k0, k1 = 128 * ch, min(128 * (ch + 1), IN_DIM)
        W1T[: k1 - k0, 128 * ch: 128 * ch + HID] = w1t[k0:k1]
    W2T = np.ascontiguousarray(
        W2.reshape(1, HID).T).astype(ml_dtypes.bfloat16)   # [128, 1]
    b1c = np.asarray(b1, dtype=np.float32).reshape(HID, 1)
    iota = np.tile(np.arange(P, dtype=ml_dtypes.bfloat16), (P, 1))

    in_maps = []
    host_masks = []
    for c in range(NCORES):
        zp = np.zeros((Npad, IN_DIM), dtype=np.float32)
        colidx = np.full(Npad, -1.0, dtype=np.float32)
        mask = np.zeros(Npad, dtype=np.float32)
        for w in range(2):
            win = cores[c][w]
            base = T[0] * P if w else 0
            n = win["n_hi"] - win["n_lo"]
            zp[base: base + n] = z[win["n_lo"]: win["n_hi"]]
            colidx[base: base + n] = (
                batch_index[win["n_lo"]: win["n_hi"]] - win["g_lo"]
            ).astype(np.float32)
            mask[base: base + n] = 1.0

        # feature-major swizzle for the MLP (512-node superchunks):
        # zth01[s, p, 512c+j] = zp[512s+j, 128c+p]; zth2 = features 256:320
        zq = zp.reshape(nSc, 512, IN_DIM).transpose(0, 2, 1)  # s, f, j
        zth01 = np.ascontiguousarray(
            zq[:, 0:256].reshape(nSc, 2, P, 512).transpose(0, 2, 1, 3)
            .reshape(nSc, P, 1024)).astype(ml_dtypes.bfloat16)
        zth2 = np.ascontiguousarray(
            zq[:, 256:320]).astype(ml_dtypes.bfloat16)        # [nSc, 64, 512]

        # node-major bf16 of pooled columns for the segment matmul,
        # superchunk-major so the quad DMA reads 2KB runs per partition:
        # znm[s, p, 512c+256j+d] = zp[512s+256c+128j+p, d]
        znm = zp[:, :POOL].reshape(nSc, 2, 2, P, POOL).transpose(
            0, 3, 1, 2, 4).reshape(nSc, P, 1024).astype(ml_dtypes.bfloat16)

        in_maps.append({
            "zth01": zth01, "zth2": zth2, "znm": np.ascontiguousarray(znm),
            "colidx": np.ascontiguousarray(colidx.reshape(nT, P).T),
            "w1t": W1T, "w2t": W2T, "b1": b1c, "iota": iota,
            "b2s": np.full((P, 1), b2s, dtype=np.float32),
        })
        host_masks.append(np.ascontiguousarray(mask.reshape(nT, P).T))
    return in_maps, host_masks


def _build_program(T):
    key = tuple(T)
    if key in _BUILD_CACHE:
        return _BUILD_CACHE[key]
    nT = T[0] + T[1]
    nCh = nT // 2
    nSc = nT // 4
    nGrp = -(-nSc // SCG)

    nc = bacc.Bacc("TRN2", target_bir_lowering=False, debug=False,
                   num_devices=NCORES)
    zth01_d = nc.dram_tensor("zth01", [nSc, P, 1024], BF16,
                             kind="ExternalInput").ap()
    zth2_d = nc.dram_tensor("zth2", [nSc, 64, 512], BF16,
                            kind="ExternalInput").ap()
    znm_d = nc.dram_tensor("znm", [nSc, P, 1024], BF16,
                           kind="ExternalInput").ap()
    colidx_d = nc.dram_tensor("colidx", [P, nT], F32, kind="ExternalInput").ap()
    b2s_d = nc.dram_tensor("b2s", [P, 1], F32, kind="ExternalInput").ap()
    w1t_d = nc.dram_tensor("w1t", [P, 384], BF16, kind="ExternalInput").ap()
    w2t_d = nc.dram_tensor("w2t", [HID, 1], BF16, kind="ExternalInput").ap()
    b1_d = nc.dram_tensor("b1", [HID, 1], F32, kind="ExternalInput").ap()
    iota_d = nc.dram_tensor("iota", [P, P], BF16, kind="ExternalInput").ap()
    out_d = nc.dram_tensor("out", [2 * P, POOL], F32, kind="ExternalOutput").ap()
    wall_d = nc.dram_tensor("wall", [P, nT], F32, kind="ExternalOutput").ap()

    with tile.TileContext(nc) as tc:
        with tc.tile_pool(name="const", bufs=1) as cpool, \
             tc.tile_pool(name="zth", bufs=6) as zthpool, \
             tc.tile_pool(name="znmp", bufs=6) as znmpool, \
             tc.tile_pool(name="hs", bufs=8) as hspool, \
             tc.tile_pool(name="oh", bufs=6) as ohpool, \
             tc.tile_pool(name="fin", bufs=1) as fpool, \
             tc.tile_pool(name="ps_h", bufs=SCG, space="PSUM") as psh, \
             tc.tile_pool(name="ps_s", bufs=2, space="PSUM") as pss, \
             tc.tile_pool(name="ps_B", bufs=1, space="PSUM") as psB:

            nch = [T[0] // 2, T[1] // 2]
            Bacc = [[psB.tile([P, POOL], F32, tag=f"B{w}a{a}",
                              name=f"B{w}a{a}")
                     for a in range(1)] for w in range(2)]

            # ---- HAM warm-up: ~5us of dummy accumulating matmuls on a
            # zeroed SBUF tile so the PE clock-gate opens before the real
            # stream; the garbage lands in Bacc[0][0], whose first real
            # matmul (start=True) resets it. ----
            warm_sb = cpool.tile([P, 256], BF16)
            nc.gpsimd.memset(warm_sb[:], 0.0)
            for i in range(NWARM):
                nc.tensor.matmul(Bacc[0][0][:], warm_sb[:, 0:128], warm_sb[:],
                                 start=(i == 0), stop=(i == NWARM - 1))

            # ---- constants ----
            w1t_sb = cpool.tile([P, 384], BF16)
            nc.gpsimd.dma_start(w1t_sb[:], w1t_d[:])
            w2t_sb = cpool.tile([HID, 1], BF16)
            nc.gpsimd.dma_start(w2t_sb[:], w2t_d[:])
            b1_sb = cpool.tile([HID, 1], F32)
            nc.gpsimd.dma_start(b1_sb[:], b1_d[:])
            iota_sb = cpool.tile([P, P], BF16)
            nc.gpsimd.dma_start(iota_sb[:], iota_d[:])
            colidx_sb = cpool.tile([P, nT], F32)
            nc.gpsimd.dma_start(colidx_sb[:], colidx_d[:])
            b2s_sb = cpool.tile([P, 1], F32)
            nc.gpsimd.dma_start(b2s_sb[:], b2s_d[:])
            w_all = cpool.tile([P, nT], F32)

            def dma_grp(k):
                sq0 = SCG * k
                np_ = min(SCG, nSc - sq0)
                zthp = zthpool.tile([P, np_ * 1024], BF16, tag="zthp",
                                    name="zthp")
                nc.sync.dma_start(
                    zthp[:].rearrange("p (a b) -> p a b", a=np_),
                    zth01_d[sq0: sq0 + np_].rearrange("a p b -> p a b"))
                zth2p = zthpool.tile([64, np_ * 512], BF16, tag="zth2p",
                                     name="zth2p")
                nc.sync.dma_start(
                    zth2p[:].rearrange("p (a b) -> p a b", a=np_),
                    zth2_d[sq0: sq0 + np_].rearrange("a p b -> p a b"))
                znmp = znmpool.tile([P, np_ * 1024], BF16, tag="znmp",
                                    name="znmp")
                nc.gpsimd.dma_start(
                    znmp[:].rearrange("p (a b) -> p a b", a=np_),
                    znm_d[sq0: sq0 + np_].rearrange("a p b -> p a b"))
                return zthp, zth2p, znmp

            def drain_window(w):
                # copy the window's PSUM accumulators to SBUF and DMA out
                accs = Bacc[w]
                tot = fpool.tile([P, POOL], F32, tag=f"comb{w}",
                                 name=f"comb{w}")
                nc.vector.tensor_copy(tot[:], accs[0][:])
                for a in range(1, len(accs)):
                    nc.vector.tensor_tensor(tot[:], tot[:], accs[a][:],
                                            ALU.add)
                nc.sync.dma_start(out_d[P * w: P * (w + 1), :], tot[:])

            bufs = {0: dma_grp(0)}
            if nGrp > 1:
                bufs[1] = dma_grp(1)

            drained = [False, False]
            wall_sent = [False, False]
            prev = None  # (superchunk ids, hs tiles, znmp tile, group idx)
            for k in range(nGrp + 1):
                if k < nGrp:
                    if k + 2 < nGrp:
                        bufs[k + 2] = dma_grp(k + 2)
                    zthp, zth2p, znmp = bufs.pop(k)
                    sq0 = SCG * k
                    np_ = min(SCG, nSc - sq0)

                # ---- scores + w = exp(s+b2) for the previous group ----
                if prev is not None:
                    psqs, phs, _, _ = prev
                    for idx, sq in enumerate(psqs):
                        s_ps = pss.tile([P, 4], F32, tag="s",
                                        name=f"sps{sq % 2}")
                        for j in range(4):
                            nc.tensor.matmul(
                                s_ps[:, j: j + 1],
                                phs[idx][:, 128 * j: 128 * (j + 1)],
                                w2t_sb[:], start=True, stop=True)
                        with tc.high_priority(offset=64):
                            nc.scalar.activation(w_all[:, 4 * sq: 4 * sq + 4],
                                                 s_ps[:], AF.Exp,
                                                 bias=b2s_sb[:])
                    # window-0 w columns all written -> ship them mid-loop
                    if psqs[-1] >= T[0] // 4 - 1 and not wall_sent[0]:
                        nc.sync.dma_start(wall_d[:, 0: T[0]],
                                          w_all[:, 0: T[0]])
                        wall_sent[0] = True

                # ---- MLP for group k ----
                if k < nGrp:
                    hps = [psh.tile([P, 512], F32, tag="h",
                                    name=f"hps{(sq0 + i) % SCG}")
                           for i in range(np_)]
                    for ch, a, b in ((0, 0, 512), (1, 512, 1024)):
                        for i in range(np_):
                            nc.tensor.matmul(
                                hps[i][:], w1t_sb[:, 128 * ch: 128 * (ch + 1)],
                                zthp[:, 1024 * i + a: 1024 * i + b],
                                start=(ch == 0), stop=False)
                    for i in range(np_):
                        nc.tensor.matmul(hps[i][:], w1t_sb[0:64, 256:384],
                                         zth2p[:, 512 * i: 512 * (i + 1)],
                                         start=False, stop=True)
                    hs = []
                    for i in range(np_):
                        h_sb = hspool.tile([P, 512], BF16, tag="hs",
                                           name=f"hs{(sq0 + i) % SCG}")
                        with tc.high_priority(offset=64):
                            nc.scalar.activation(h_sb[:], hps[i][:], AF.Relu,
                                                 bias=b1_sb[:])
                        hs.append(h_sb)

                # ---- one-hot + segment matmuls for the previous group ----
                if prev is not None:
                    psqs, phs, pznmp, pk = prev
                    for lq in range(2 * len(psqs)):
                        q = 2 * SCG * pk + lq
                        w = 0 if q < T[0] // 2 else 1
                        qw = q - (T[0] // 2 if w else 0)
                        oh = ohpool.tile([P, 256], BF16, tag="oh", name="oh")
                        for j in (0, 1):
                            t = 2 * q + j
                            with tc.high_priority(offset=64):
                                nc.vector.tensor_scalar(
                                    oh[:, 128 * j: 128 * (j + 1)], iota_sb[:],
                                    colidx_sb[:, t: t + 1], w_all[:, t: t + 1],
                                    ALU.is_equal, ALU.mult)
                        na = len(Bacc[w])
                        acc = Bacc[w][qw % na]
                        sl, c2 = lq // 2, lq % 2
                        zo = 1024 * sl + 512 * c2
                        for j in (0, 1):
                            nc.tensor.matmul(
                                acc[:], oh[:, 128 * j: 128 * (j + 1)],
                                pznmp[:, zo + 256 * j: zo + 256 * (j + 1)],
                                start=(qw < na and j == 0),
                                stop=(qw + na >= nch[w] and j == 1))
                        # window finished -> ship its sums mid-loop
                        if qw == nch[w] - 1 and not drained[w]:
                            drain_window(w)
                            drained[w] = True

                # keep-warm fillers: the loop is DMA-bound (~8us/group)
                # while the warm PE only needs ~5.5us, so without filler
                # the PE idles >3.4us and the HAM re-throttles it to
                # 1.2GHz (which then makes the PE the bottleneck).  Dummy
                # matmuls plug the idle below the HAM window.  They land
                # in whichever window accumulator is not live: window 1's
                # before its first real chunk, window 0's after its drain.
                g_w1_first = (T[0] // 2) // (2 * SCG)      # group of w1's 1st chunk
                g_w0_last = (T[0] // 2 - 1) // (2 * SCG)   # group of w0's last chunk
                nfill = 18 if k <= 2 else 8
                fb = None
                if 1 <= k <= g_w1_first:       # seg processed here is group k-1
                    fb = Bacc[1][0]
                elif g_w0_last + 2 <= k < nGrp:  # w0 drained in iter g_w0_last+1
                    fb = Bacc[0][0]
                if fb is not None and T[0] // 2 > 2 * SCG * 3:
                    for i in range(nfill):
                        nc.tensor.matmul(fb[:], warm_sb[:, 0:128],
                                         warm_sb[:], start=(i == 0),
                                         stop=(i == nfill - 1))

                if k < nGrp:
                    prev = ([sq0 + i for i in range(np_)], hs, znmp, k)
                else:
                    prev = None

            # ---- remaining w columns (window 1) ----
            nc.sync.dma_start(wall_d[:, T[0]: nT], w_all[:, T[0]: nT])
            for w in range(2):
                if not drained[w]:
                    drain_window(w)

    nc.compile()
    _BUILD_CACHE[key] = nc
    return nc


def kernel(z, batch_index, W1, b1, W2, b2, num_graphs):
    global LAST_RESULT
    z = np.asarray(z, dtype=np.float32)
    batch_index = np.asarray(batch_index)
    G = int(num_graphs)

    counts, cores, T = _plan(batch_index, G)
    in_maps, host_masks = _build_inputs(
        z, batch_index, np.asarray(W1), np.asarray(b1),
        np.asarray(W2), np.asarray(b2), counts, cores, T)
    nc = _build_program(T)

    res = run_bass_kernel_spmd(
        nc, in_maps, list(range(NCORES)),
        trace=PROFILE["trace"],
        **({"tmpdir": PROFILE["tmpdir"]} if PROFILE["tmpdir"] else {}))
    LAST_RESULT = res

    # host-side finish: global softmax denominator (pads masked here) and
    # the per-graph scaling by 1/(S * count)
    S = 0.0
    for c in range(NCORES):
        wall = np.asarray(res.results[c]["wall"], np.float64)
        S += float((wall * host_masks[c]).sum())
    out = np.zeros((G, POOL), dtype=np.float32)
    for c in range(NCORES):
        for w in range(2):
            win = cores[c][w]
            ng = win["g_hi"] - win["g_lo"]
            if ng:
                B = np.asarray(res.results[c]["out"][P * w: P * w + ng],
                               np.float64)
                cnt = np.maximum(counts[win["g_lo"]: win["g_hi"]],
                                 1).astype(np.float64)
                out[win["g_lo"]: win["g_hi"]] = \
                    (B / (S * cnt[:, None])).astype(np.float32)
    return out
